# revision 16
# baseline (speedup 1.0000x reference)
"""Trainium2 Bass kernel for nn_AttentionBase (RoPE multi-head attention + out-proj).

Sharding: 8 cores, core c handles batch b=c//2 and query-row half c%2.
Each core computes all 16 heads for its 1024 query rows against the full
2048 keys/values of its batch, so softmax and the output projection are
entirely local to the core (no collectives). Host slices inputs / concats
outputs.

Device layout (per core):
  - q/k are fed transposed per head-pair: [pair, 128, n] where rows
    0-31/32-63 are head 2p's even/odd rope lanes and 64-127 head 2p+1.
    (The d-permutation cancels in q.k and is matched by v/wT ordering.)
  - RoPE: partition-swap via SBUF->SBUF DMA + 3 vector ops with
    host-precomputed cos / (+-sin) tables.
  - scores^T[m, n] = k_rot @ q_rot^T  (fp32r matmuls, 2 heads row-packed)
  - exp on ScalarE (scale=1/8 fused); no max-subtraction needed (|s|<~7).
  - out^T accumulated in PSUM with both heads col-packed (rows 0-63/64-127)
  - denominators via ones-vector matmuls, division deferred to one
    tensor-tensor divide per pair.
  - projection: out[n,e] accumulated over head-pairs (K=128), bias folded
    in as a rank-1 (K=1) matmul.
"""

import sys

if "/opt/trn_rl_repo" not in sys.path:
    sys.path.insert(0, "/opt/trn_rl_repo")

import numpy as np

import concourse.bass as bass
import concourse.tile as tile
from concourse import bacc
from concourse import mybir
from concourse.bass_utils import run_bass_kernel_spmd

F32 = mybir.dt.float32
F32R = mybir.dt.float32r

NUM_HEADS = 16
HEAD_DIM = 64
ROPE_BASE = 10000.0
B = 4
N = 2048
MID = NUM_HEADS * HEAD_DIM
N_CORES = 8

# full-size kernel configuration
FULL_CFG = dict(n_pairs=8, n_mc=16, n_halves=2)
NT = 512  # n-tile width (fp32 matmul moving-dim max)


def build_nc(n_pairs=8, n_mc=16, n_halves=2, repeat=1):
    """Build the (single-program SPMD) Bass module."""
    nt = NT
    nq = n_halves * nt  # local query rows
    m = n_mc * 128  # keys
    n_heads = 2 * n_pairs
    e_dim = n_heads * HEAD_DIM  # projection output dim == mid
    n_ec = (e_dim + 511) // 512
    n_nc2 = nt // 128
    scale = HEAD_DIM**-0.5

    nc = bacc.Bacc("TRN2", target_bir_lowering=False)
    qT_d = nc.dram_tensor("qT", [n_pairs, 128, nq], F32R, kind="ExternalInput").ap()
    kT_d = nc.dram_tensor("kT", [n_pairs, 128, m], F32R, kind="ExternalInput").ap()
    qTs_d = nc.dram_tensor("qTs", [n_pairs, 128, nq], F32R, kind="ExternalInput").ap()
    kTs_d = nc.dram_tensor("kTs", [n_pairs, 128, m], F32R, kind="ExternalInput").ap()
    v_d = nc.dram_tensor(
        "v", [n_heads, n_mc, 128, 128], F32R, kind="ExternalInput"
    ).ap()
    wT_d = nc.dram_tensor("wT", [n_pairs, 128, e_dim], F32R, kind="ExternalInput").ap()
    bias_d = nc.dram_tensor("bias", [1, e_dim], F32R, kind="ExternalInput").ap()
    onesrow_d = nc.dram_tensor("onesrow", [1, 128], F32R, kind="ExternalInput").ap()
    cq_d = nc.dram_tensor("cq", [128, nq], F32R, kind="ExternalInput").ap()
    sq_d = nc.dram_tensor("sq", [128, nq], F32R, kind="ExternalInput").ap()
    ck_d = nc.dram_tensor("ck", [128, m], F32R, kind="ExternalInput").ap()
    sk_d = nc.dram_tensor("sk", [128, m], F32R, kind="ExternalInput").ap()
    out_d = nc.dram_tensor("out", [nq, e_dim], F32, kind="ExternalOutput").ap()

    with tile.TileContext(nc) as tc:
        with (
            tc.tile_pool(name="singles", bufs=1) as singles,
            tc.tile_pool(name="rotq", bufs=n_pairs) as rotq_pool,
            tc.tile_pool(name="rotk", bufs=n_pairs) as rotk_pool,
        ):
            onesrow = singles.tile([1, 128], F32R)
            nc.sync.dma_start(out=onesrow, in_=onesrow_d)
            bias_sb = singles.tile([1, e_dim], F32R)
            nc.sync.dma_start(out=bias_sb, in_=bias_d)
            wts = []
            for p in range(n_pairs):
                wt = singles.tile([128, e_dim], F32R, name=f"wt{p}", tag=f"wt{p}")
                nc.sync.dma_start(out=wt, in_=wT_d[p])
                wts.append(wt)

            def one_pass():
                # ---- RoPE phase: rotate q and k into persistent SBUF tiles ----
                qrots = []
                krots = []
                with (
                    tc.tile_pool(name="tables", bufs=1) as tables,
                    tc.tile_pool(name="ropetmp", bufs=1) as ropetmp,
                ):
                    cqt = tables.tile([128, nq], F32R)
                    nc.sync.dma_start(out=cqt, in_=cq_d)
                    sqt = tables.tile([128, nq], F32R)
                    nc.sync.dma_start(out=sqt, in_=sq_d)
                    ckt = tables.tile([128, m], F32R)
                    nc.sync.dma_start(out=ckt, in_=ck_d)
                    skt = tables.tile([128, m], F32R)
                    nc.sync.dma_start(out=skt, in_=sk_d)

                    def rope(dst_pool, src_dram, src_sw_dram, cos_t, sin_t, width, tagsuf, idx):
                        dst = dst_pool.tile(
                            [128, width], F32R, name=f"rot{tagsuf}{idx}", tag=f"rot{tagsuf}"
                        )
                        nc.sync.dma_start(out=dst, in_=src_dram)
                        sw = ropetmp.tile([128, width], F32R, name=f"sw{tagsuf}", tag=f"sw{tagsuf}")
                        nc.sync.dma_start(out=sw, in_=src_sw_dram)
                        t0 = ropetmp.tile([128, width], F32R, name=f"t0{tagsuf}", tag=f"t0{tagsuf}")
                        nc.vector.tensor_mul(t0, dst, cos_t)
                        nc.vector.tensor_mul(sw, sw, sin_t)
                        nc.vector.tensor_add(dst, sw, t0)
                        return dst

                    for p in range(n_pairs):
                        qrots.append(rope(rotq_pool, qT_d[p], qTs_d[p], cqt, sqt, nq, "q", p))
                        krots.append(rope(rotk_pool, kT_d[p], kTs_d[p], ckt, skt, m, "k", p))

                # ---- attention + projection ----
                with (
                    tc.tile_pool(name="scp", bufs=2, space="PSUM") as scp,
                    tc.tile_pool(name="avp", bufs=3, space="PSUM") as avp,
                    tc.tile_pool(name="pjp", bufs=1, space="PSUM") as pjp,
                    tc.tile_pool(name="ep", bufs=3) as ep,
                    tc.tile_pool(name="vp", bufs=4) as vp,
                    tc.tile_pool(name="stp", bufs=n_pairs) as stp,
                    tc.tile_pool(name="avcp", bufs=2) as avcp,
                    tc.tile_pool(name="denp", bufs=2) as denp,
                    tc.tile_pool(name="recp", bufs=2) as recp,
                    tc.tile_pool(name="obp", bufs=2) as obp,
                ):
                    for nh in range(n_halves):
                        stages = []
                        for p in range(n_pairs):
                            # psA: [outU_A(0:64); denA(64:128)], psB: [denB; outU_B]
                            psA = avp.tile([128, nt], F32, name="psA", tag="av")
                            psB = avp.tile([128, nt], F32, name="psB", tag="av")
                            for mc in range(n_mc):
                                sc = scp.tile([128, 2 * nt], F32, tag="sc")
                                for h2 in range(2):
                                    lhsT = krots[p][
                                        h2 * 64 : (h2 + 1) * 64, mc * 128 : (mc + 1) * 128
                                    ]
                                    rhs = qrots[p][
                                        h2 * 64 : (h2 + 1) * 64, nh * nt : (nh + 1) * nt
                                    ]
                                    nc.tensor.matmul(
                                        sc[:, h2 * nt : (h2 + 1) * nt],
                                        lhsT=lhsT,
                                        rhs=rhs,
                                        start=True,
                                        stop=True,
                                    )
                                et_t = ep.tile([128, 2 * nt], F32R, tag="et")
                                nc.scalar.activation(
                                    et_t,
                                    sc,
                                    mybir.ActivationFunctionType.Exp,
                                    scale=scale,
                                )
                                for h2, ps in ((0, psA), (1, psB)):
                                    vt = vp.tile(
                                        [128, 128], F32R, name=f"vt{h2}", tag=f"vt{h2}"
                                    )
                                    nc.sync.dma_start(out=vt, in_=v_d[2 * p + h2, mc])
                                    nc.tensor.matmul(
                                        ps,
                                        lhsT=vt,
                                        rhs=et_t[:, h2 * nt : (h2 + 1) * nt],
                                        start=(mc == 0),
                                        stop=(mc == n_mc - 1),
                                        skip_group_check=True,
                                    )
                            # assemble numerator [outU_A; outU_B] lane-aligned
                            avc = avcp.tile([128, nt], F32, tag="avc")
                            nc.vector.tensor_copy(out=avc[0:64, :], in_=psA[0:64, :])
                            nc.vector.tensor_copy(out=avc[64:128, :], in_=psB[64:128, :])
                            # denominators: [denB(0:64); denA(64:128)], recip, then
                            # partition-swap via SBUF->SBUF DMA to align with avc
                            den = denp.tile([128, nt], F32, tag="den")
                            nc.vector.tensor_copy(out=den[0:64, :], in_=psB[0:64, :])
                            nc.vector.tensor_copy(out=den[64:128, :], in_=psA[64:128, :])
                            nc.vector.reciprocal(out=den, in_=den)
                            rec = recp.tile([128, nt], F32, tag="rec")
                            nc.sync.dma_start(out=rec[0:64, :], in_=den[64:128, :])
                            nc.sync.dma_start(out=rec[64:128, :], in_=den[0:64, :])
                            stg = stp.tile([128, nt], F32R, name=f"stg{p}", tag="stg")
                            nc.vector.tensor_mul(stg, avc, rec)
                            stages.append(stg)
                        for nc2 in range(n_nc2):
                            for ec in range(n_ec):
                                ecs = min(512, e_dim - ec * 512)
                                pj = pjp.tile([128, ecs], F32, tag="pj")
                                for p in range(n_pairs):
                                    nc.tensor.matmul(
                                        pj,
                                        lhsT=stages[p][:, nc2 * 128 : (nc2 + 1) * 128],
                                        rhs=wts[p][:, ec * 512 : ec * 512 + ecs].bitcast(F32R),
                                        start=(p == 0),
                                        stop=False,
                                        skip_group_check=True,
                                    )
                                nc.tensor.matmul(
                                    pj,
                                    lhsT=onesrow,
                                    rhs=bias_sb[0:1, ec * 512 : ec * 512 + ecs].bitcast(F32R),
                                    start=False,
                                    stop=True,
                                    skip_group_check=True,
                                )
                                ob = obp.tile([128, ecs], F32, tag="ob")
                                nc.vector.tensor_copy(out=ob, in_=pj)
                                nc.sync.dma_start(
                                    out=out_d[
                                        nh * nt + nc2 * 128 : nh * nt + (nc2 + 1) * 128,
                                        ec * 512 : ec * 512 + ecs,
                                    ],
                                    in_=ob,
                                )

            for _rep in range(repeat):
                one_pass()
    nc.finalize()
    return nc


def _sin_cos_np(positions, dim=HEAD_DIM):
    """fp32 sin/cos tables matching reference._sin_cos numerics."""
    inv_freq = (
        1.0 / (ROPE_BASE ** (np.arange(0, dim, 2, dtype=np.float32) / np.float32(dim)))
    ).astype(np.float32)
    ang = positions.astype(np.float32)[:, None] * inv_freq[None, :]
    return np.sin(ang).astype(np.float32), np.cos(ang).astype(np.float32)


def _pack_T(x, n_pairs):
    """[n, n_heads*64] -> [n_pairs, 128, n] with per-head even/odd d split."""
    n = x.shape[0]
    n_heads = 2 * n_pairs
    xr = x.reshape(n, n_heads, 32, 2)
    # [heads, 2(even/odd), 32, n]
    stk = np.ascontiguousarray(xr.transpose(1, 3, 2, 0))
    return stk.reshape(n_pairs, 128, n).astype(np.float32)


def _rope_tables(positions, n_blocks=4):
    """cos table [128, n] (cos tiled 4x) and sign-folded sin table [-s;s;-s;s]."""
    sin, cos = _sin_cos_np(positions)
    cosT = cos.T  # [32, n]
    sinT = sin.T
    c = np.tile(cosT, (n_blocks, 1)).astype(np.float32)
    s = np.concatenate([-sinT, sinT] * (n_blocks // 2), axis=0).astype(np.float32)
    return c, s


def prep_core_inputs(q_slice, k_full, v_full, w_out, b_out, q_positions, k_positions):
    """Build the per-core DRAM input dict (full-size config)."""
    n_pairs = NUM_HEADS // 2
    n_mc = k_full.shape[0] // 128
    n_heads = NUM_HEADS
    qT = _pack_T(q_slice, n_pairs)
    kT = _pack_T(k_full, n_pairs)
    # v: [m, heads*64] -> [heads, n_mc, 128, 128]; even head [v|1], odd [1|v]
    m = k_full.shape[0]
    v_r = v_full.reshape(m, n_heads, HEAD_DIM).transpose(1, 0, 2)
    v_r = np.ascontiguousarray(v_r).reshape(n_heads, n_mc, 128, HEAD_DIM)
    v1 = np.ones((n_heads, n_mc, 128, 128), np.float32)
    v1[0::2, :, :, 0:HEAD_DIM] = v_r[0::2]
    v1[1::2, :, :, HEAD_DIM:] = v_r[1::2]
    v_r = v1
    wT = np.ascontiguousarray(w_out.T).reshape(n_pairs, 128, n_heads * HEAD_DIM)
    cq, sq = _rope_tables(q_positions)
    ck, sk = _rope_tables(k_positions)
    perm = np.r_[32:64, 0:32, 96:128, 64:96]
    return {
        "qT": qT.astype(np.float32),
        "kT": kT.astype(np.float32),
        "qTs": np.ascontiguousarray(qT[:, perm, :]).astype(np.float32),
        "kTs": np.ascontiguousarray(kT[:, perm, :]).astype(np.float32),
        "v": v_r.astype(np.float32),
        "wT": wT.astype(np.float32),
        "bias": b_out.reshape(1, -1).astype(np.float32),
        "onesrow": np.ones((1, 128), np.float32),
        "cq": cq,
        "sq": sq,
        "ck": ck,
        "sk": sk,
    }


_NC_CACHE = {}


def _get_nc(repeat=1):
    key = ("full", repeat)
    if key not in _NC_CACHE:
        _NC_CACHE[key] = build_nc(**FULL_CFG, repeat=repeat)
    return _NC_CACHE[key]


class _Runner:
    """Cached jitted SPMD executor for a Bass module (mirrors
    bass2jax.run_bass_via_pjrt, but reusable across calls so the NEFF is
    compiled once and timing can exclude host transfers)."""

    def __init__(self, nc, n_cores):
        import jax
        from jax.experimental.shard_map import shard_map
        from jax.sharding import Mesh, NamedSharding, PartitionSpec

        from concourse import mybir as _mybir
        from concourse.bass2jax import (
            _bass_exec_p,
            install_neuronx_cc_hook,
            partition_id_tensor,
        )

        install_neuronx_cc_hook()
        self.nc = nc
        self.n_cores = n_cores
        partition_name = (
            nc.partition_id_tensor.name if nc.partition_id_tensor else None
        )
        in_names = []
        out_names = []
        out_avals = []
        for alloc in nc.m.functions[0].allocations:
            if not isinstance(alloc, _mybir.MemoryLocationSet):
                continue
            name = alloc.memorylocations[0].name
            if alloc.kind == "ExternalInput":
                if name != partition_name:
                    in_names.append(name)
            elif alloc.kind == "ExternalOutput":
                shape = tuple(alloc.tensor_shape)
                dtype = _mybir.dt.np(alloc.dtype)
                out_names.append(name)
                out_avals.append(jax.core.ShapedArray(shape, dtype))
        self.in_names = in_names
        self.out_names = out_names
        self.out_avals = out_avals
        n_params = len(in_names)
        n_outs = len(out_names)
        all_names = in_names + out_names
        if partition_name is not None:
            all_names.append(partition_name)
        donate = tuple(range(n_params, n_params + n_outs))

        def _body(*args):
            operands = list(args)
            if partition_name is not None:
                operands.append(partition_id_tensor())
            outs = _bass_exec_p.bind(
                *operands,
                out_avals=tuple(out_avals),
                in_names=tuple(all_names),
                out_names=tuple(out_names),
                lowering_input_output_aliases=(),
                sim_require_finite=True,
                sim_require_nnan=True,
                nc=nc,
            )
            return tuple(outs)

        devices = jax.devices()[:n_cores]
        assert len(devices) == n_cores
        self.mesh = Mesh(np.asarray(devices), ("core",))
        self.sharding = NamedSharding(self.mesh, PartitionSpec("core"))
        in_specs = (PartitionSpec("core"),) * (n_params + n_outs)
        out_specs = (PartitionSpec("core"),) * n_outs
        self.jitted = jax.jit(
            shard_map(
                _body,
                mesh=self.mesh,
                in_specs=in_specs,
                out_specs=out_specs,
                check_rep=False,
            ),
            donate_argnums=donate,
            keep_unused=True,
        )
        self._jax = jax

    def concat_inputs(self, in_maps):
        return [
            np.concatenate([np.asarray(m[name]) for m in in_maps], axis=0)
            for name in self.in_names
        ]

    def zeros(self):
        return [
            np.zeros((self.n_cores * a.shape[0], *a.shape[1:]), a.dtype)
            for a in self.out_avals
        ]

    def device_put_inputs(self, concat_in):
        return [self._jax.device_put(x, self.sharding) for x in concat_in]

    def device_put_zeros(self):
        return [self._jax.device_put(z, self.sharding) for z in self.zeros()]

    def run_device(self, dev_in, dev_zeros):
        """Execute with device-resident args; returns device arrays."""
        return self.jitted(*dev_in, *dev_zeros)

    def run(self, in_maps):
        dev_in = self.device_put_inputs(self.concat_inputs(in_maps))
        outs = self.run_device(dev_in, self.device_put_zeros())
        return [
            {
                name: np.asarray(outs[i]).reshape(
                    self.n_cores, *self.out_avals[i].shape
                )[c]
                for i, name in enumerate(self.out_names)
            }
            for c in range(self.n_cores)
        ]


_RUNNER_CACHE = {}


def _get_runner(repeat=1):
    if repeat not in _RUNNER_CACHE:
        _RUNNER_CACHE[repeat] = _Runner(_get_nc(repeat), N_CORES)
    return _RUNNER_CACHE[repeat]


def make_in_maps(q, k, v, w_out, b_out):
    q = np.asarray(q, dtype=np.float32)
    k = np.asarray(k, dtype=np.float32)
    v = np.asarray(v, dtype=np.float32)
    w_out = np.asarray(w_out, dtype=np.float32)
    b_out = np.asarray(b_out, dtype=np.float32)
    nq = N // 2
    in_maps = []
    for c in range(N_CORES):
        b = c // 2
        ns = (c % 2) * nq
        in_maps.append(
            prep_core_inputs(
                q[b, ns : ns + nq],
                k[b],
                v[b],
                w_out,
                b_out,
                np.arange(ns, ns + nq),
                np.arange(N),
            )
        )
    return in_maps


def gather_out(results):
    nq = N // 2
    out = np.empty((B, N, MID), dtype=np.float32)
    for c in range(N_CORES):
        b = c // 2
        ns = (c % 2) * nq
        out[b, ns : ns + nq, :] = results[c]["out"]
    return out


def kernel(q, k, v, w_out, b_out):
    runner = _get_runner()
    results = runner.run(make_in_maps(q, k, v, w_out, b_out))
    return gather_out(results)


# revision 18
# speedup vs baseline: 1.0545x; 1.0545x over previous
"""Trainium2 Bass kernel for nn_AttentionBase (RoPE multi-head attention + out-proj).

Sharding: 8 cores, core c handles batch b=c//2 and query-row half c%2.
Each core computes all 16 heads for its 1024 query rows against the full
2048 keys/values of its batch, so softmax and the output projection are
entirely local to the core (no collectives). Host slices inputs / concats
outputs.

Device layout (per core):
  - q/k are fed transposed per head-pair: [pair, 128, n] where rows
    0-31/32-63 are head 2p's even/odd rope lanes and 64-127 head 2p+1.
    (The d-permutation cancels in q.k and is matched by v/wT ordering.)
  - RoPE: partition-swap via SBUF->SBUF DMA + 3 vector ops with
    host-precomputed cos / (+-sin) tables.
  - scores^T[m, n] = k_rot @ q_rot^T  (fp32r matmuls, 2 heads row-packed)
  - exp on ScalarE (scale=1/8 fused); no max-subtraction needed (|s|<~7).
  - out^T accumulated in PSUM with both heads col-packed (rows 0-63/64-127)
  - denominators via ones-vector matmuls, division deferred to one
    tensor-tensor divide per pair.
  - projection: out[n,e] accumulated over head-pairs (K=128), bias folded
    in as a rank-1 (K=1) matmul.
"""

import sys

if "/opt/trn_rl_repo" not in sys.path:
    sys.path.insert(0, "/opt/trn_rl_repo")

import numpy as np

import concourse.bass as bass
import concourse.tile as tile
from concourse import bacc
from concourse import mybir
from concourse.bass_utils import run_bass_kernel_spmd

F32 = mybir.dt.float32
F32R = mybir.dt.float32r

NUM_HEADS = 16
HEAD_DIM = 64
ROPE_BASE = 10000.0
B = 4
N = 2048
MID = NUM_HEADS * HEAD_DIM
N_CORES = 8

# full-size kernel configuration
FULL_CFG = dict(n_pairs=8, n_mc=16, n_halves=2)
NT = 512  # n-tile width (fp32 matmul moving-dim max)


def build_nc(n_pairs=8, n_mc=16, n_halves=2, repeat=1, level=3):
    """Build the (single-program SPMD) Bass module."""
    nt = NT
    nq = n_halves * nt  # local query rows
    m = n_mc * 128  # keys
    n_heads = 2 * n_pairs
    e_dim = n_heads * HEAD_DIM  # projection output dim == mid
    n_ec = (e_dim + 511) // 512
    n_nc2 = nt // 128
    scale = HEAD_DIM**-0.5

    nc = bacc.Bacc("TRN2", target_bir_lowering=False)
    qT_d = nc.dram_tensor("qT", [n_pairs, 128, nq], F32R, kind="ExternalInput").ap()
    kT_d = nc.dram_tensor("kT", [n_pairs, 128, m], F32R, kind="ExternalInput").ap()
    qTs_d = nc.dram_tensor("qTs", [n_pairs, 128, nq], F32R, kind="ExternalInput").ap()
    kTs_d = nc.dram_tensor("kTs", [n_pairs, 128, m], F32R, kind="ExternalInput").ap()
    v_d = nc.dram_tensor(
        "v", [n_heads, n_mc, 128, HEAD_DIM], F32R, kind="ExternalInput"
    ).ap()
    ones64_d = nc.dram_tensor("ones64", [128, 64], F32R, kind="ExternalInput").ap()
    wT_d = nc.dram_tensor("wT", [n_pairs, 128, e_dim], F32R, kind="ExternalInput").ap()
    bias_d = nc.dram_tensor("bias", [1, e_dim], F32R, kind="ExternalInput").ap()
    onesrow_d = nc.dram_tensor("onesrow", [1, 128], F32R, kind="ExternalInput").ap()
    cq_d = nc.dram_tensor("cq", [128, nq], F32R, kind="ExternalInput").ap()
    sq_d = nc.dram_tensor("sq", [128, nq], F32R, kind="ExternalInput").ap()
    ck_d = nc.dram_tensor("ck", [128, m], F32R, kind="ExternalInput").ap()
    sk_d = nc.dram_tensor("sk", [128, m], F32R, kind="ExternalInput").ap()
    out_d = nc.dram_tensor("out", [nq, e_dim], F32, kind="ExternalOutput").ap()

    with tile.TileContext(nc) as tc:
        with (
            tc.tile_pool(name="singles", bufs=1) as singles,
            tc.tile_pool(name="rotq", bufs=n_pairs) as rotq_pool,
            tc.tile_pool(name="rotk", bufs=n_pairs) as rotk_pool,
        ):
            onesrow = singles.tile([1, 128], F32R)
            nc.sync.dma_start(out=onesrow, in_=onesrow_d)
            bias_sb = singles.tile([1, e_dim], F32R)
            nc.sync.dma_start(out=bias_sb, in_=bias_d)
            wts = []
            for p in range(n_pairs):
                wt = singles.tile([128, e_dim], F32R, name=f"wt{p}", tag=f"wt{p}")
                nc.sync.dma_start(out=wt, in_=wT_d[p])
                wts.append(wt)
            # persistent v-tile rings: [v|1] for even heads, [1|v] for odd
            NRING = 4
            vringA = []
            vringB = []
            for i in range(NRING):
                va = singles.tile([128, 128], F32R, name=f"vra{i}", tag=f"vra{i}")
                nc.sync.dma_start(out=va[:, 64:128], in_=ones64_d)
                vringA.append(va)
                vb = singles.tile([128, 128], F32R, name=f"vrb{i}", tag=f"vrb{i}")
                nc.sync.dma_start(out=vb[:, 0:64], in_=ones64_d)
                vringB.append(vb)

            def one_pass():
                # ---- RoPE phase: rotate q and k into persistent SBUF tiles ----
                qrots = []
                krots = []
                with (
                    tc.tile_pool(name="tables", bufs=1) as tables,
                    tc.tile_pool(name="ropetmp", bufs=1) as ropetmp,
                ):
                    cqt = tables.tile([128, nq], F32R)
                    nc.sync.dma_start(out=cqt, in_=cq_d)
                    sqt = tables.tile([128, nq], F32R)
                    nc.sync.dma_start(out=sqt, in_=sq_d)
                    ckt = tables.tile([128, m], F32R)
                    nc.sync.dma_start(out=ckt, in_=ck_d)
                    skt = tables.tile([128, m], F32R)
                    nc.sync.dma_start(out=skt, in_=sk_d)

                    def rope(dst_pool, src_dram, src_sw_dram, cos_t, sin_t, width, tagsuf, idx):
                        dst = dst_pool.tile(
                            [128, width], F32R, name=f"rot{tagsuf}{idx}", tag=f"rot{tagsuf}"
                        )
                        nc.sync.dma_start(out=dst, in_=src_dram)
                        sw = ropetmp.tile([128, width], F32R, name=f"sw{tagsuf}", tag=f"sw{tagsuf}")
                        nc.sync.dma_start(out=sw, in_=src_sw_dram)
                        t0 = ropetmp.tile([128, width], F32R, name=f"t0{tagsuf}", tag=f"t0{tagsuf}")
                        nc.vector.tensor_mul(t0, dst, cos_t)
                        nc.vector.tensor_mul(sw, sw, sin_t)
                        nc.vector.tensor_add(dst, sw, t0)
                        return dst

                    for p in range(n_pairs):
                        qrots.append(rope(rotq_pool, qT_d[p], qTs_d[p], cqt, sqt, nq, "q", p))
                        krots.append(rope(rotk_pool, kT_d[p], kTs_d[p], ckt, skt, m, "k", p))

                # ---- attention + projection ----
                with (
                    tc.tile_pool(name="scp", bufs=2, space="PSUM") as scp,
                    tc.tile_pool(name="avp", bufs=3, space="PSUM") as avp,
                    tc.tile_pool(name="pjp", bufs=1, space="PSUM") as pjp,
                    tc.tile_pool(name="ep", bufs=3) as ep,
                    tc.tile_pool(name="stp", bufs=n_pairs) as stp,
                    tc.tile_pool(name="avcp", bufs=2) as avcp,
                    tc.tile_pool(name="denp", bufs=2) as denp,
                    tc.tile_pool(name="recp", bufs=2) as recp,
                    tc.tile_pool(name="obp", bufs=2) as obp,
                ):
                    for nh in range(n_halves):
                        stages = []
                        for p in range(n_pairs):
                            # psA: [outU_A(0:64); denA(64:128)], psB: [denB; outU_B]
                            psA = avp.tile([128, nt], F32, name="psA", tag="av")
                            psB = avp.tile([128, nt], F32, name="psB", tag="av")
                            def emit_av(mc, et_prev):
                                slot = mc % NRING
                                for h2, ps, vt in (
                                    (0, psA, vringA[slot]),
                                    (1, psB, vringB[slot]),
                                ):
                                    nc.tensor.matmul(
                                        ps,
                                        lhsT=vt,
                                        rhs=et_prev[:, h2 * nt : (h2 + 1) * nt],
                                        start=(mc == 0),
                                        stop=(mc == n_mc - 1),
                                        skip_group_check=True,
                                    )

                            pending = None
                            for mc in range(n_mc):
                                sc = scp.tile([128, 2 * nt], F32, tag="sc")
                                for h2 in range(2):
                                    lhsT = krots[p][
                                        h2 * 64 : (h2 + 1) * 64, mc * 128 : (mc + 1) * 128
                                    ]
                                    rhs = qrots[p][
                                        h2 * 64 : (h2 + 1) * 64, nh * nt : (nh + 1) * nt
                                    ]
                                    nc.tensor.matmul(
                                        sc[:, h2 * nt : (h2 + 1) * nt],
                                        lhsT=lhsT,
                                        rhs=rhs,
                                        start=True,
                                        stop=True,
                                    )
                                if level < 1:
                                    continue
                                # prefetch v chunks for this mc into the ring
                                if level >= 2:
                                    slot = mc % NRING
                                    nc.sync.dma_start(
                                        out=vringA[slot][:, 0:64],
                                        in_=v_d[2 * p + 0, mc],
                                    )
                                    nc.sync.dma_start(
                                        out=vringB[slot][:, 64:128],
                                        in_=v_d[2 * p + 1, mc],
                                    )
                                et_t = ep.tile([128, 2 * nt], F32R, tag="et")
                                nc.scalar.activation(
                                    et_t,
                                    sc,
                                    mybir.ActivationFunctionType.Exp,
                                    scale=scale,
                                )
                                if level < 2:
                                    continue
                                if pending is not None:
                                    emit_av(*pending)
                                pending = (mc, et_t)
                            if pending is not None:
                                emit_av(*pending)
                            if level < 3:
                                continue
                            # assemble numerator [outU_A; outU_B] lane-aligned
                            avc = avcp.tile([128, nt], F32, tag="avc")
                            nc.vector.tensor_copy(out=avc[0:64, :], in_=psA[0:64, :])
                            nc.vector.tensor_copy(out=avc[64:128, :], in_=psB[64:128, :])
                            # denominators: [denB(0:64); denA(64:128)], recip, then
                            # partition-swap via SBUF->SBUF DMA to align with avc
                            den = denp.tile([128, nt], F32, tag="den")
                            nc.vector.tensor_copy(out=den[0:64, :], in_=psB[0:64, :])
                            nc.vector.tensor_copy(out=den[64:128, :], in_=psA[64:128, :])
                            nc.vector.reciprocal(out=den, in_=den)
                            rec = recp.tile([128, nt], F32, tag="rec")
                            nc.sync.dma_start(out=rec[0:64, :], in_=den[64:128, :])
                            nc.sync.dma_start(out=rec[64:128, :], in_=den[0:64, :])
                            stg = stp.tile([128, nt], F32R, name=f"stg{p}", tag="stg")
                            nc.vector.tensor_mul(stg, avc, rec)
                            stages.append(stg)
                        if level < 3:
                            continue
                        for nc2 in range(n_nc2):
                            for ec in range(n_ec):
                                ecs = min(512, e_dim - ec * 512)
                                pj = pjp.tile([128, ecs], F32, tag="pj")
                                for p in range(n_pairs):
                                    nc.tensor.matmul(
                                        pj,
                                        lhsT=stages[p][:, nc2 * 128 : (nc2 + 1) * 128],
                                        rhs=wts[p][:, ec * 512 : ec * 512 + ecs].bitcast(F32R),
                                        start=(p == 0),
                                        stop=False,
                                        skip_group_check=True,
                                    )
                                nc.tensor.matmul(
                                    pj,
                                    lhsT=onesrow,
                                    rhs=bias_sb[0:1, ec * 512 : ec * 512 + ecs].bitcast(F32R),
                                    start=False,
                                    stop=True,
                                    skip_group_check=True,
                                )
                                ob = obp.tile([128, ecs], F32, tag="ob")
                                nc.vector.tensor_copy(out=ob, in_=pj)
                                nc.sync.dma_start(
                                    out=out_d[
                                        nh * nt + nc2 * 128 : nh * nt + (nc2 + 1) * 128,
                                        ec * 512 : ec * 512 + ecs,
                                    ],
                                    in_=ob,
                                )

            for _rep in range(repeat):
                one_pass()
    nc.finalize()
    return nc


def _sin_cos_np(positions, dim=HEAD_DIM):
    """fp32 sin/cos tables matching reference._sin_cos numerics."""
    inv_freq = (
        1.0 / (ROPE_BASE ** (np.arange(0, dim, 2, dtype=np.float32) / np.float32(dim)))
    ).astype(np.float32)
    ang = positions.astype(np.float32)[:, None] * inv_freq[None, :]
    return np.sin(ang).astype(np.float32), np.cos(ang).astype(np.float32)


def _pack_T(x, n_pairs):
    """[n, n_heads*64] -> [n_pairs, 128, n] with per-head even/odd d split."""
    n = x.shape[0]
    n_heads = 2 * n_pairs
    xr = x.reshape(n, n_heads, 32, 2)
    # [heads, 2(even/odd), 32, n]
    stk = np.ascontiguousarray(xr.transpose(1, 3, 2, 0))
    return stk.reshape(n_pairs, 128, n).astype(np.float32)


def _rope_tables(positions, n_blocks=4):
    """cos table [128, n] (cos tiled 4x) and sign-folded sin table [-s;s;-s;s]."""
    sin, cos = _sin_cos_np(positions)
    cosT = cos.T  # [32, n]
    sinT = sin.T
    c = np.tile(cosT, (n_blocks, 1)).astype(np.float32)
    s = np.concatenate([-sinT, sinT] * (n_blocks // 2), axis=0).astype(np.float32)
    return c, s


def prep_core_inputs(q_slice, k_full, v_full, w_out, b_out, q_positions, k_positions):
    """Build the per-core DRAM input dict (full-size config)."""
    n_pairs = NUM_HEADS // 2
    n_mc = k_full.shape[0] // 128
    n_heads = NUM_HEADS
    qT = _pack_T(q_slice, n_pairs)
    kT = _pack_T(k_full, n_pairs)
    # v: [m, heads*64] -> [heads, n_mc, 128, 64]
    m = k_full.shape[0]
    v_r = v_full.reshape(m, n_heads, HEAD_DIM).transpose(1, 0, 2)
    v_r = np.ascontiguousarray(v_r).reshape(n_heads, n_mc, 128, HEAD_DIM)
    wT = np.ascontiguousarray(w_out.T).reshape(n_pairs, 128, n_heads * HEAD_DIM)
    cq, sq = _rope_tables(q_positions)
    ck, sk = _rope_tables(k_positions)
    perm = np.r_[32:64, 0:32, 96:128, 64:96]
    return {
        "qT": qT.astype(np.float32),
        "kT": kT.astype(np.float32),
        "qTs": np.ascontiguousarray(qT[:, perm, :]).astype(np.float32),
        "kTs": np.ascontiguousarray(kT[:, perm, :]).astype(np.float32),
        "v": v_r.astype(np.float32),
        "wT": wT.astype(np.float32),
        "bias": b_out.reshape(1, -1).astype(np.float32),
        "ones64": np.ones((128, 64), np.float32),
        "onesrow": np.ones((1, 128), np.float32),
        "cq": cq,
        "sq": sq,
        "ck": ck,
        "sk": sk,
    }


_NC_CACHE = {}


def _get_nc(repeat=1, level=3):
    key = ("full", repeat, level)
    if key not in _NC_CACHE:
        _NC_CACHE[key] = build_nc(**FULL_CFG, repeat=repeat, level=level)
    return _NC_CACHE[key]


class _Runner:
    """Cached jitted SPMD executor for a Bass module (mirrors
    bass2jax.run_bass_via_pjrt, but reusable across calls so the NEFF is
    compiled once and timing can exclude host transfers)."""

    def __init__(self, nc, n_cores):
        import jax
        from jax.experimental.shard_map import shard_map
        from jax.sharding import Mesh, NamedSharding, PartitionSpec

        from concourse import mybir as _mybir
        from concourse.bass2jax import (
            _bass_exec_p,
            install_neuronx_cc_hook,
            partition_id_tensor,
        )

        install_neuronx_cc_hook()
        self.nc = nc
        self.n_cores = n_cores
        partition_name = (
            nc.partition_id_tensor.name if nc.partition_id_tensor else None
        )
        in_names = []
        out_names = []
        out_avals = []
        for alloc in nc.m.functions[0].allocations:
            if not isinstance(alloc, _mybir.MemoryLocationSet):
                continue
            name = alloc.memorylocations[0].name
            if alloc.kind == "ExternalInput":
                if name != partition_name:
                    in_names.append(name)
            elif alloc.kind == "ExternalOutput":
                shape = tuple(alloc.tensor_shape)
                dtype = _mybir.dt.np(alloc.dtype)
                out_names.append(name)
                out_avals.append(jax.core.ShapedArray(shape, dtype))
        self.in_names = in_names
        self.out_names = out_names
        self.out_avals = out_avals
        n_params = len(in_names)
        n_outs = len(out_names)
        all_names = in_names + out_names
        if partition_name is not None:
            all_names.append(partition_name)
        donate = tuple(range(n_params, n_params + n_outs))

        def _body(*args):
            operands = list(args)
            if partition_name is not None:
                operands.append(partition_id_tensor())
            outs = _bass_exec_p.bind(
                *operands,
                out_avals=tuple(out_avals),
                in_names=tuple(all_names),
                out_names=tuple(out_names),
                lowering_input_output_aliases=(),
                sim_require_finite=True,
                sim_require_nnan=True,
                nc=nc,
            )
            return tuple(outs)

        devices = jax.devices()[:n_cores]
        assert len(devices) == n_cores
        self.mesh = Mesh(np.asarray(devices), ("core",))
        self.sharding = NamedSharding(self.mesh, PartitionSpec("core"))
        in_specs = (PartitionSpec("core"),) * (n_params + n_outs)
        out_specs = (PartitionSpec("core"),) * n_outs
        self.jitted = jax.jit(
            shard_map(
                _body,
                mesh=self.mesh,
                in_specs=in_specs,
                out_specs=out_specs,
                check_rep=False,
            ),
            donate_argnums=donate,
            keep_unused=True,
        )
        self._jax = jax

    def concat_inputs(self, in_maps):
        return [
            np.concatenate([np.asarray(m[name]) for m in in_maps], axis=0)
            for name in self.in_names
        ]

    def zeros(self):
        return [
            np.zeros((self.n_cores * a.shape[0], *a.shape[1:]), a.dtype)
            for a in self.out_avals
        ]

    def device_put_inputs(self, concat_in):
        return [self._jax.device_put(x, self.sharding) for x in concat_in]

    def device_put_zeros(self):
        return [self._jax.device_put(z, self.sharding) for z in self.zeros()]

    def run_device(self, dev_in, dev_zeros):
        """Execute with device-resident args; returns device arrays."""
        return self.jitted(*dev_in, *dev_zeros)

    def run(self, in_maps):
        dev_in = self.device_put_inputs(self.concat_inputs(in_maps))
        outs = self.run_device(dev_in, self.device_put_zeros())
        return [
            {
                name: np.asarray(outs[i]).reshape(
                    self.n_cores, *self.out_avals[i].shape
                )[c]
                for i, name in enumerate(self.out_names)
            }
            for c in range(self.n_cores)
        ]


_RUNNER_CACHE = {}


def _get_runner(repeat=1, level=3):
    key = (repeat, level)
    if key not in _RUNNER_CACHE:
        _RUNNER_CACHE[key] = _Runner(_get_nc(repeat, level), N_CORES)
    return _RUNNER_CACHE[key]


def make_in_maps(q, k, v, w_out, b_out):
    q = np.asarray(q, dtype=np.float32)
    k = np.asarray(k, dtype=np.float32)
    v = np.asarray(v, dtype=np.float32)
    w_out = np.asarray(w_out, dtype=np.float32)
    b_out = np.asarray(b_out, dtype=np.float32)
    nq = N // 2
    in_maps = []
    for c in range(N_CORES):
        b = c // 2
        ns = (c % 2) * nq
        in_maps.append(
            prep_core_inputs(
                q[b, ns : ns + nq],
                k[b],
                v[b],
                w_out,
                b_out,
                np.arange(ns, ns + nq),
                np.arange(N),
            )
        )
    return in_maps


def gather_out(results):
    nq = N // 2
    out = np.empty((B, N, MID), dtype=np.float32)
    for c in range(N_CORES):
        b = c // 2
        ns = (c % 2) * nq
        out[b, ns : ns + nq, :] = results[c]["out"]
    return out


def kernel(q, k, v, w_out, b_out):
    runner = _get_runner()
    results = runner.run(make_in_maps(q, k, v, w_out, b_out))
    return gather_out(results)
